# revision 3
# baseline (speedup 1.0000x reference)
"""Trainium2 Bass kernel for the gated-recurrence module.

Math (per reference):
    g      = sigmoid(uV2) * tanh(uV_raw)                       # [HD]
    A      = einsum('tbi,hi->tbh', x, WA)                      # [T,B,HD]
    per step k (sequential over T):
        V_k = X_k + g * Q_{k-1}                                # [B,ID]
        FR  = V_k @ Wih.T                                      # [B,2HD]
        F   = sigmoid(FR[:, :HD] + PFB)
        R   = sigmoid(FR[:, HD:] + HWb)
        S   = F*S + (1-F)*A_k
        Q   = R*S + (1-R)*X_k        -> Y[k] = Q
    returns (Y, Q_final[None], S_final[None])

Distribution: data-parallel over batch B=64 across 8 cores (8 per core).

Device layout ("transposed world"): every [B, HD] tensor lives as
[128 partitions, (h_tile, b) free] where h = 128*h_tile + p.  The per-step
GEMM uses stationary weight tiles (Wih^T [128,128] blocks) and streams
V^T [128, 8].  Gate biases are injected into PSUM by one extra matmul
(lhsT = bias reshaped [8,128] zero-padded, rhs = kron(I8, ones(1,8))), so
each gate bank needs a single sigmoid ACT op over [128, 64].
"""

import os
import sys

import numpy as np

for _p in ("/opt/trn_rl_repo", "/opt/pypackages"):
    if _p not in sys.path:
        sys.path.append(_p)

T, B, HD = 512, 64, 1024
ID = HD
NCORES = 8
BL = B // NCORES          # batch per core = 8
KT = ID // 128            # contraction tiles = 8
C = 16                    # recurrence chunk length (steps)
ACH = 64                  # A-phase chunk length (steps)

LAST_RESULTS = None       # BassKernelResults of the most recent run (for test.py)


def _build_program(t_steps=T):
    import concourse.bass as bass
    import concourse.mybir as mybir
    import concourse.tile as tile
    from concourse import bacc
    from contextlib import ExitStack

    f32 = mybir.dt.float32
    SIG = mybir.ActivationFunctionType.Sigmoid
    COPY = mybir.ActivationFunctionType.Copy

    nc = bacc.Bacc("TRN2", target_bir_lowering=False, debug=False, num_devices=NCORES)

    x_p = nc.dram_tensor("x_p", [KT, 128, t_steps, BL], f32, kind="ExternalInput").ap()
    wih = nc.dram_tensor("wih", [128, KT, 2 * HD], f32, kind="ExternalInput").ap()
    wa = nc.dram_tensor("wa", [128, KT, HD], f32, kind="ExternalInput").ap()
    gbv = nc.dram_tensor("gbv", [128, KT, BL], f32, kind="ExternalInput").ap()
    bf = nc.dram_tensor("bf", [128, 128], f32, kind="ExternalInput").ap()
    br = nc.dram_tensor("br", [128, 128], f32, kind="ExternalInput").ap()
    ind = nc.dram_tensor("ind", [128, KT * BL], f32, kind="ExternalInput").ap()
    q0 = nc.dram_tensor("q0", [128, KT, BL], f32, kind="ExternalInput").ap()
    s0 = nc.dram_tensor("s0", [128, KT, BL], f32, kind="ExternalInput").ap()
    y_p = nc.dram_tensor("y_p", [KT, 128, t_steps, BL], f32, kind="ExternalOutput").ap()
    sf = nc.dram_tensor("sf", [128, KT, BL], f32, kind="ExternalOutput").ap()

    with tile.TileContext(nc) as tc, ExitStack() as ctx:
        constp = ctx.enter_context(tc.tile_pool(name="const", bufs=1))
        dramp = ctx.enter_context(tc.tile_pool(name="dram", bufs=1, space="DRAM"))
        xap = ctx.enter_context(tc.tile_pool(name="xa", bufs=2))
        apsum = ctx.enter_context(tc.tile_pool(name="apsum", bufs=2, space="PSUM"))
        acp = ctx.enter_context(tc.tile_pool(name="acopy", bufs=3))
        xrp = ctx.enter_context(tc.tile_pool(name="xr", bufs=2))
        arp = ctx.enter_context(tc.tile_pool(name="ar", bufs=2))
        yp = ctx.enter_context(tc.tile_pool(name="y", bufs=2))
        rpsum = ctx.enter_context(tc.tile_pool(name="rpsum", bufs=2, space="PSUM"))
        frp = ctx.enter_context(tc.tile_pool(name="fr", bufs=2))
        vp = ctx.enter_context(tc.tile_pool(name="v", bufs=2))
        tmpp = ctx.enter_context(tc.tile_pool(name="tmp", bufs=3))

        # ---- constants ----
        wih_sb = constp.tile([128, KT, 2 * HD], f32)
        nc.sync.dma_start(wih_sb[:], wih)
        wa_sb = constp.tile([128, KT, HD], f32)
        nc.sync.dma_start(wa_sb[:], wa)
        gb_sb = constp.tile([128, KT, BL], f32)
        nc.sync.dma_start(gb_sb[:], gbv)
        bf_sb = constp.tile([128, 128], f32)
        nc.sync.dma_start(bf_sb[:], bf)
        br_sb = constp.tile([128, 128], f32)
        nc.sync.dma_start(br_sb[:], br)
        ind_sb = constp.tile([128, KT * BL], f32)
        nc.sync.dma_start(ind_sb[:], ind)
        q0_sb = constp.tile([128, KT, BL], f32)
        nc.sync.dma_start(q0_sb[:], q0)
        S = constp.tile([128, KT, BL], f32)
        nc.sync.dma_start(S[:], s0)

        a_dram = dramp.tile([KT, 128, t_steps, BL], f32)

        # ---- Phase A: A^T = WA @ X^T  (big parallel GEMM) ----
        ach = min(ACH, t_steps)
        for ac in range(t_steps // ach):
            xc = xap.tile([128, KT, ach, BL], f32, tag="xa")
            nc.sync.dma_start(
                xc[:],
                x_p[:, :, ac * ach:(ac + 1) * ach, :].rearrange("i p k b -> p i k b"),
            )
            for m in range(KT):
                ps = apsum.tile([128, ach * BL], f32, tag="aps")
                for i in range(KT):
                    nc.tensor.matmul(
                        ps[:],
                        wa_sb[:, i, m * 128:(m + 1) * 128],
                        xc[:, i],
                        start=(i == 0),
                        stop=(i == KT - 1),
                    )
                asb = acp.tile([128, ach, BL], f32, tag="acp")
                nc.scalar.activation(asb.rearrange("p k b -> p (k b)"), ps[:], COPY)
                nc.sync.dma_start(a_dram[m, :, ac * ach:(ac + 1) * ach, :], asb[:])

        # ---- Phase R: the recurrence ----
        prev_q = q0_sb[:]
        for ch in range(t_steps // C):
            xc = xrp.tile([128, KT, C, BL], f32, tag="xr")
            nc.sync.dma_start(
                xc[:],
                x_p[:, :, ch * C:(ch + 1) * C, :].rearrange("i p k b -> p i k b"),
            )
            aq = arp.tile([128, KT, C, BL], f32, tag="ar")
            nc.sync.dma_start(
                aq[:],
                a_dram[:, :, ch * C:(ch + 1) * C, :].rearrange("i p k b -> p i k b"),
            )
            yc = yp.tile([128, KT, C, BL], f32, tag="y")
            for kk in range(C):
                Xk = xc[:, :, kk, :]          # [128, KT, BL] strided
                Ak = aq[:, :, kk, :]
                # V = Xk + gb * Q_prev
                V = vp.tile([128, KT, BL], f32, tag="v")
                nc.vector.tensor_mul(V[:], gb_sb[:], prev_q)
                nc.vector.tensor_add(V[:], V[:], Xk)
                # D = S - A  (early; off the post-sigmoid critical path)
                D = tmpp.tile([128, KT, BL], f32, tag="d")
                nc.vector.tensor_sub(D[:], S[:], Ak)
                # gates GEMM: psF gets rows [0,1024), psR gets rows [1024,2048)
                psF = rpsum.tile([128, KT * BL], f32, tag="psF")
                psR = rpsum.tile([128, KT * BL], f32, tag="psR")
                nc.tensor.matmul(psF[:], bf_sb[:], ind_sb[:], start=True, stop=False,
                                 skip_group_check=True)
                for j in range(8):
                    for i in range(KT):
                        nc.tensor.matmul(
                            psF[:, j * BL:(j + 1) * BL],
                            wih_sb[:, i, j * 128:(j + 1) * 128],
                            V[:, i, :],
                            start=False,
                            stop=(j == 7 and i == KT - 1),
                            skip_group_check=True,
                        )
                F = frp.tile([128, KT * BL], f32, tag="F")
                nc.scalar.activation(F[:], psF[:], SIG)
                F3 = F.rearrange("p (i b) -> p i b", i=KT)
                # S = F*D + A
                nc.vector.tensor_mul(S[:], F3, D[:])
                nc.vector.tensor_add(S[:], S[:], Ak)
                # E = S - X (needs new S; still overlaps the R half of the GEMM)
                E = tmpp.tile([128, KT, BL], f32, tag="e")
                nc.vector.tensor_sub(E[:], S[:], Xk)
                nc.tensor.matmul(psR[:], br_sb[:], ind_sb[:], start=True, stop=False,
                                 skip_group_check=True)
                for j in range(8):
                    for i in range(KT):
                        nc.tensor.matmul(
                            psR[:, j * BL:(j + 1) * BL],
                            wih_sb[:, i, (8 + j) * 128:(9 + j) * 128],
                            V[:, i, :],
                            start=False,
                            stop=(j == 7 and i == KT - 1),
                            skip_group_check=True,
                        )
                R = frp.tile([128, KT * BL], f32, tag="R")
                nc.scalar.activation(R[:], psR[:], SIG)
                R3 = R.rearrange("p (i b) -> p i b", i=KT)
                # Q = R*E + X  -> written straight into the Y chunk
                Qs = yc[:, :, kk, :]
                nc.vector.tensor_mul(Qs, R3, E[:])
                nc.vector.tensor_add(Qs, Qs, Xk)
                prev_q = Qs
            nc.sync.dma_start(
                y_p[:, :, ch * C:(ch + 1) * C, :].rearrange("i p k b -> p i k b"),
                yc[:],
            )
        nc.sync.dma_start(sf, S[:])

    nc.compile()
    return nc


_PROGRAM_CACHE = {}


def _get_program(t_steps=T):
    if t_steps not in _PROGRAM_CACHE:
        _PROGRAM_CACHE[t_steps] = _build_program(t_steps)
    return _PROGRAM_CACHE[t_steps]


def kernel(x, Q0, S0, Wih, WA, uV_raw, uV2, PFB, HWb):
    global LAST_RESULTS
    from concourse.bass_utils import run_bass_kernel_spmd

    x = np.ascontiguousarray(np.asarray(x, dtype=np.float32))
    Q0 = np.asarray(Q0, dtype=np.float32)
    S0 = np.asarray(S0, dtype=np.float32)
    Wih = np.asarray(Wih, dtype=np.float32)
    WA = np.asarray(WA, dtype=np.float32)
    uV_raw = np.asarray(uV_raw, dtype=np.float32)
    uV2 = np.asarray(uV2, dtype=np.float32)
    PFB = np.asarray(PFB, dtype=np.float32)
    HWb = np.asarray(HWb, dtype=np.float32)

    t_steps = x.shape[0]

    # ---- host-side packing (pure layout, no FLOPs except the tiny [HD] gate) ----
    g = (1.0 / (1.0 + np.exp(-uV2, dtype=np.float64)) * np.tanh(uV_raw,
         dtype=np.float64)).astype(np.float32)                     # [HD]
    gb_packed = np.ascontiguousarray(
        np.broadcast_to(g.reshape(KT, 128).T[:, :, None], (128, KT, BL))
    ).astype(np.float32)

    wih_packed = np.ascontiguousarray(
        Wih.T.reshape(KT, 128, 2 * HD).transpose(1, 0, 2)
    )                                                              # [128, KT, 2HD]
    wa_packed = np.ascontiguousarray(
        WA.T.reshape(KT, 128, HD).transpose(1, 0, 2)
    )                                                              # [128, KT, HD]

    bf_packed = np.zeros((128, 128), np.float32)
    bf_packed[:KT] = PFB.reshape(KT, 128)
    br_packed = np.zeros((128, 128), np.float32)
    br_packed[:KT] = HWb.reshape(KT, 128)
    ind_packed = np.zeros((128, KT * BL), np.float32)
    ind_packed[:KT] = np.kron(np.eye(KT, dtype=np.float32), np.ones((1, BL), np.float32))

    shared = {
        "wih": wih_packed,
        "wa": wa_packed,
        "gbv": gb_packed,
        "bf": bf_packed,
        "br": br_packed,
        "ind": ind_packed,
    }

    in_maps = []
    for c in range(NCORES):
        bsl = slice(c * BL, (c + 1) * BL)
        x_core = x[:, bsl, :]                                      # [T, BL, HD]
        x_packed = np.ascontiguousarray(
            x_core.reshape(t_steps, BL, KT, 128).transpose(2, 3, 0, 1)
        )                                                          # [KT, 128, T, BL]
        q0_packed = np.ascontiguousarray(
            Q0[0, bsl, :].reshape(BL, KT, 128).transpose(2, 1, 0)
        )                                                          # [128, KT, BL]
        s0_packed = np.ascontiguousarray(
            S0[0, bsl, :].reshape(BL, KT, 128).transpose(2, 1, 0)
        )
        in_maps.append(dict(shared, x_p=x_packed, q0=q0_packed, s0=s0_packed))

    nc = _get_program(t_steps)
    trace = bool(int(os.environ.get("BK_TRACE", "0")))
    res = run_bass_kernel_spmd(nc, in_maps, core_ids=list(range(NCORES)), trace=trace)
    LAST_RESULTS = res

    # ---- unpack ----
    Y = np.empty((t_steps, B, HD), np.float32)
    Sf = np.empty((1, B, HD), np.float32)
    for c in range(NCORES):
        bsl = slice(c * BL, (c + 1) * BL)
        y_core = np.asarray(res.results[c]["y_p"])                 # [KT, 128, T, BL]
        Y[:, bsl, :] = y_core.transpose(2, 3, 0, 1).reshape(t_steps, BL, HD)
        sf_core = np.asarray(res.results[c]["sf"])                 # [128, KT, BL]
        Sf[0, bsl, :] = sf_core.transpose(2, 1, 0).reshape(BL, HD)
    Qf = Y[-1][None].copy()
    return Y, Qf, Sf


# revision 4
# speedup vs baseline: 8.1806x; 8.1806x over previous
"""Trainium2 Bass kernel for the gated-recurrence module.

Math (per reference):
    g      = sigmoid(uV2) * tanh(uV_raw)                       # [HD]
    A      = einsum('tbi,hi->tbh', x, WA)                      # [T,B,HD]
    per step k (sequential over T):
        V_k = X_k + g * Q_{k-1}                                # [B,ID]
        FR  = V_k @ Wih.T                                      # [B,2HD]
        F   = sigmoid(FR[:, :HD] + PFB)
        R   = sigmoid(FR[:, HD:] + HWb)
        S   = F*S + (1-F)*A_k
        Q   = R*S + (1-R)*X_k        -> Y[k] = Q
    returns (Y, Q_final[None], S_final[None])

Distribution: data-parallel over batch B=64 across 8 cores (8 per core).

Device layout ("transposed world"): every [B, HD] tensor lives as
[128 partitions, (h_tile, b) free] where h = 128*h_tile + p.  The per-step
GEMM uses stationary weight tiles (Wih^T [128,128] blocks) and streams
V^T [128, 8].  Gate biases are injected into PSUM by one extra matmul
(lhsT = bias reshaped [8,128] zero-padded, rhs = kron(I8, ones(1,8))), so
each gate bank needs a single sigmoid ACT op over [128, 64].
"""

import os
import sys

import numpy as np

for _p in ("/opt/trn_rl_repo", "/opt/pypackages"):
    if _p not in sys.path:
        sys.path.append(_p)

T, B, HD = 512, 64, 1024
ID = HD
NCORES = 8
BL = B // NCORES          # batch per core = 8
KT = ID // 128            # contraction tiles = 8
C = 16                    # recurrence chunk length (steps)
ACH = 64                  # A-phase chunk length (steps)

LAST_RESULTS = None       # BassKernelResults of the most recent run (for test.py)


def _build_program(t_steps=T):
    import concourse.bass as bass
    import concourse.mybir as mybir
    import concourse.tile as tile
    from concourse import bacc
    from contextlib import ExitStack

    f32 = mybir.dt.float32
    bf16 = mybir.dt.bfloat16
    SIG = mybir.ActivationFunctionType.Sigmoid
    COPY = mybir.ActivationFunctionType.Copy

    nc = bacc.Bacc("TRN2", target_bir_lowering=False, debug=False, num_devices=NCORES)

    x_p = nc.dram_tensor("x_p", [KT, 128, t_steps, BL], f32, kind="ExternalInput").ap()
    wih = nc.dram_tensor("wih", [128, KT, 2 * HD], bf16, kind="ExternalInput").ap()
    wa = nc.dram_tensor("wa", [128, KT, HD], f32, kind="ExternalInput").ap()
    gbv = nc.dram_tensor("gbv", [128, KT, BL], f32, kind="ExternalInput").ap()
    bf = nc.dram_tensor("bf", [128, 128], bf16, kind="ExternalInput").ap()
    br = nc.dram_tensor("br", [128, 128], bf16, kind="ExternalInput").ap()
    ind = nc.dram_tensor("ind", [128, KT * BL], bf16, kind="ExternalInput").ap()
    q0 = nc.dram_tensor("q0", [128, KT, BL], f32, kind="ExternalInput").ap()
    s0 = nc.dram_tensor("s0", [128, KT, BL], f32, kind="ExternalInput").ap()
    y_p = nc.dram_tensor("y_p", [KT, 128, t_steps, BL], f32, kind="ExternalOutput").ap()
    sf = nc.dram_tensor("sf", [128, KT, BL], f32, kind="ExternalOutput").ap()

    with tile.TileContext(nc) as tc, ExitStack() as ctx:
        constp = ctx.enter_context(tc.tile_pool(name="const", bufs=1))
        dramp = ctx.enter_context(tc.tile_pool(name="dram", bufs=1, space="DRAM"))
        xap = ctx.enter_context(tc.tile_pool(name="xa", bufs=2))
        apsum = ctx.enter_context(tc.tile_pool(name="apsum", bufs=2, space="PSUM"))
        acp = ctx.enter_context(tc.tile_pool(name="acopy", bufs=3))
        xrp = ctx.enter_context(tc.tile_pool(name="xr", bufs=2))
        arp = ctx.enter_context(tc.tile_pool(name="ar", bufs=2))
        yp = ctx.enter_context(tc.tile_pool(name="y", bufs=2))
        rpsum = ctx.enter_context(tc.tile_pool(name="rpsum", bufs=2, space="PSUM"))
        frp = ctx.enter_context(tc.tile_pool(name="fr", bufs=2))
        vp = ctx.enter_context(tc.tile_pool(name="v", bufs=2))
        tmpp = ctx.enter_context(tc.tile_pool(name="tmp", bufs=3))

        # ---- constants ----
        wih_sb = constp.tile([128, KT, 2 * HD], bf16)
        nc.sync.dma_start(wih_sb[:], wih)
        wa_sb = constp.tile([128, KT, HD], f32)
        nc.sync.dma_start(wa_sb[:], wa)
        gb_sb = constp.tile([128, KT, BL], f32)
        nc.sync.dma_start(gb_sb[:], gbv)
        bf_sb = constp.tile([128, 128], bf16)
        nc.sync.dma_start(bf_sb[:], bf)
        br_sb = constp.tile([128, 128], bf16)
        nc.sync.dma_start(br_sb[:], br)
        ind_sb = constp.tile([128, KT * BL], bf16)
        nc.sync.dma_start(ind_sb[:], ind)
        q0_sb = constp.tile([128, KT, BL], f32)
        nc.sync.dma_start(q0_sb[:], q0)
        S = constp.tile([128, KT, BL], f32)
        nc.sync.dma_start(S[:], s0)

        a_dram = dramp.tile([KT, 128, t_steps, BL], f32)

        # ---- Phase A: A^T = WA @ X^T  (big parallel GEMM) ----
        ach = min(ACH, t_steps)
        for ac in range(t_steps // ach):
            xc = xap.tile([128, KT, ach, BL], f32, tag="xa")
            nc.sync.dma_start(
                xc[:],
                x_p[:, :, ac * ach:(ac + 1) * ach, :].rearrange("i p k b -> p i k b"),
            )
            for m in range(KT):
                ps = apsum.tile([128, ach * BL], f32, tag="aps")
                for i in range(KT):
                    nc.tensor.matmul(
                        ps[:],
                        wa_sb[:, i, m * 128:(m + 1) * 128],
                        xc[:, i],
                        start=(i == 0),
                        stop=(i == KT - 1),
                    )
                asb = acp.tile([128, ach, BL], f32, tag="acp")
                nc.scalar.activation(asb.rearrange("p k b -> p (k b)"), ps[:], COPY)
                nc.sync.dma_start(a_dram[m, :, ac * ach:(ac + 1) * ach, :], asb[:])

        # ---- Phase R: the recurrence ----
        prev_q = q0_sb[:]
        for ch in range(t_steps // C):
            xc = xrp.tile([128, KT, C, BL], f32, tag="xr")
            nc.sync.dma_start(
                xc[:],
                x_p[:, :, ch * C:(ch + 1) * C, :].rearrange("i p k b -> p i k b"),
            )
            aq = arp.tile([128, KT, C, BL], f32, tag="ar")
            nc.sync.dma_start(
                aq[:],
                a_dram[:, :, ch * C:(ch + 1) * C, :].rearrange("i p k b -> p i k b"),
            )
            yc = yp.tile([128, KT, C, BL], f32, tag="y")
            for kk in range(C):
                Xk = xc[:, :, kk, :]          # [128, KT, BL] strided
                Ak = aq[:, :, kk, :]
                # V = Xk + gb * Q_prev
                V = vp.tile([128, KT, BL], bf16, tag="v")
                nc.vector.tensor_mul(V[:], gb_sb[:], prev_q)
                nc.vector.tensor_add(V[:], V[:], Xk)
                # D = S - A  (early; off the post-sigmoid critical path)
                D = tmpp.tile([128, KT, BL], f32, tag="d")
                nc.vector.tensor_sub(D[:], S[:], Ak)
                # gates GEMM: psF gets rows [0,1024), psR gets rows [1024,2048)
                psF = rpsum.tile([128, KT * BL], f32, tag="psF")
                psR = rpsum.tile([128, KT * BL], f32, tag="psR")
                nc.tensor.matmul(psF[:], bf_sb[:], ind_sb[:], start=True, stop=False,
                                 skip_group_check=True)
                for j in range(8):
                    for i in range(KT):
                        nc.tensor.matmul(
                            psF[:, j * BL:(j + 1) * BL],
                            wih_sb[:, i, j * 128:(j + 1) * 128],
                            V[:, i, :],
                            start=False,
                            stop=(j == 7 and i == KT - 1),
                            skip_group_check=True,
                        )
                F = frp.tile([128, KT * BL], f32, tag="F")
                nc.scalar.activation(F[:], psF[:], SIG)
                F3 = F.rearrange("p (i b) -> p i b", i=KT)
                # S = F*D + A
                nc.vector.tensor_mul(S[:], F3, D[:])
                nc.vector.tensor_add(S[:], S[:], Ak)
                # E = S - X (needs new S; still overlaps the R half of the GEMM)
                E = tmpp.tile([128, KT, BL], f32, tag="e")
                nc.vector.tensor_sub(E[:], S[:], Xk)
                nc.tensor.matmul(psR[:], br_sb[:], ind_sb[:], start=True, stop=False,
                                 skip_group_check=True)
                for j in range(8):
                    for i in range(KT):
                        nc.tensor.matmul(
                            psR[:, j * BL:(j + 1) * BL],
                            wih_sb[:, i, (8 + j) * 128:(9 + j) * 128],
                            V[:, i, :],
                            start=False,
                            stop=(j == 7 and i == KT - 1),
                            skip_group_check=True,
                        )
                R = frp.tile([128, KT * BL], f32, tag="R")
                nc.scalar.activation(R[:], psR[:], SIG)
                R3 = R.rearrange("p (i b) -> p i b", i=KT)
                # Q = R*E + X  -> written straight into the Y chunk
                Qs = yc[:, :, kk, :]
                nc.vector.tensor_mul(Qs, R3, E[:])
                nc.vector.tensor_add(Qs, Qs, Xk)
                prev_q = Qs
            nc.sync.dma_start(
                y_p[:, :, ch * C:(ch + 1) * C, :].rearrange("i p k b -> p i k b"),
                yc[:],
            )
        nc.sync.dma_start(sf, S[:])

    nc.compile()
    return nc


_PROGRAM_CACHE = {}


def _get_program(t_steps=T):
    if t_steps not in _PROGRAM_CACHE:
        _PROGRAM_CACHE[t_steps] = _build_program(t_steps)
    return _PROGRAM_CACHE[t_steps]


def kernel(x, Q0, S0, Wih, WA, uV_raw, uV2, PFB, HWb):
    global LAST_RESULTS
    from concourse.bass_utils import run_bass_kernel_spmd

    x = np.ascontiguousarray(np.asarray(x, dtype=np.float32))
    Q0 = np.asarray(Q0, dtype=np.float32)
    S0 = np.asarray(S0, dtype=np.float32)
    Wih = np.asarray(Wih, dtype=np.float32)
    WA = np.asarray(WA, dtype=np.float32)
    uV_raw = np.asarray(uV_raw, dtype=np.float32)
    uV2 = np.asarray(uV2, dtype=np.float32)
    PFB = np.asarray(PFB, dtype=np.float32)
    HWb = np.asarray(HWb, dtype=np.float32)

    t_steps = x.shape[0]

    # ---- host-side packing (pure layout, no FLOPs except the tiny [HD] gate) ----
    g = (1.0 / (1.0 + np.exp(-uV2, dtype=np.float64)) * np.tanh(uV_raw,
         dtype=np.float64)).astype(np.float32)                     # [HD]
    gb_packed = np.ascontiguousarray(
        np.broadcast_to(g.reshape(KT, 128).T[:, :, None], (128, KT, BL))
    ).astype(np.float32)

    import ml_dtypes
    wih_packed = np.ascontiguousarray(
        Wih.T.reshape(KT, 128, 2 * HD).transpose(1, 0, 2)
    ).astype(ml_dtypes.bfloat16)                                   # [128, KT, 2HD]
    wa_packed = np.ascontiguousarray(
        WA.T.reshape(KT, 128, HD).transpose(1, 0, 2)
    )                                                              # [128, KT, HD]

    bf_packed = np.zeros((128, 128), ml_dtypes.bfloat16)
    bf_packed[:KT] = PFB.reshape(KT, 128).astype(ml_dtypes.bfloat16)
    br_packed = np.zeros((128, 128), ml_dtypes.bfloat16)
    br_packed[:KT] = HWb.reshape(KT, 128).astype(ml_dtypes.bfloat16)
    ind_packed = np.zeros((128, KT * BL), ml_dtypes.bfloat16)
    ind_packed[:KT] = np.kron(np.eye(KT, dtype=np.float32), np.ones((1, BL), np.float32))

    shared = {
        "wih": wih_packed,
        "wa": wa_packed,
        "gbv": gb_packed,
        "bf": bf_packed,
        "br": br_packed,
        "ind": ind_packed,
    }

    in_maps = []
    for c in range(NCORES):
        bsl = slice(c * BL, (c + 1) * BL)
        x_core = x[:, bsl, :]                                      # [T, BL, HD]
        x_packed = np.ascontiguousarray(
            x_core.reshape(t_steps, BL, KT, 128).transpose(2, 3, 0, 1)
        )                                                          # [KT, 128, T, BL]
        q0_packed = np.ascontiguousarray(
            Q0[0, bsl, :].reshape(BL, KT, 128).transpose(2, 1, 0)
        )                                                          # [128, KT, BL]
        s0_packed = np.ascontiguousarray(
            S0[0, bsl, :].reshape(BL, KT, 128).transpose(2, 1, 0)
        )
        in_maps.append(dict(shared, x_p=x_packed, q0=q0_packed, s0=s0_packed))

    nc = _get_program(t_steps)
    trace = bool(int(os.environ.get("BK_TRACE", "0")))
    res = run_bass_kernel_spmd(nc, in_maps, core_ids=list(range(NCORES)), trace=trace)
    LAST_RESULTS = res

    # ---- unpack ----
    Y = np.empty((t_steps, B, HD), np.float32)
    Sf = np.empty((1, B, HD), np.float32)
    for c in range(NCORES):
        bsl = slice(c * BL, (c + 1) * BL)
        y_core = np.asarray(res.results[c]["y_p"])                 # [KT, 128, T, BL]
        Y[:, bsl, :] = y_core.transpose(2, 3, 0, 1).reshape(t_steps, BL, HD)
        sf_core = np.asarray(res.results[c]["sf"])                 # [128, KT, BL]
        Sf[0, bsl, :] = sf_core.transpose(2, 1, 0).reshape(BL, HD)
    Qf = Y[-1][None].copy()
    return Y, Qf, Sf


# revision 6
# speedup vs baseline: 9.0732x; 1.1091x over previous
"""Trainium2 Bass kernel for the gated-recurrence module.

Math (per reference):
    g      = sigmoid(uV2) * tanh(uV_raw)                       # [HD]
    A      = einsum('tbi,hi->tbh', x, WA)                      # [T,B,HD]
    per step k (sequential over T):
        V_k = X_k + g * Q_{k-1}                                # [B,ID]
        FR  = V_k @ Wih.T                                      # [B,2HD]
        F   = sigmoid(FR[:, :HD] + PFB)
        R   = sigmoid(FR[:, HD:] + HWb)
        S   = F*S + (1-F)*A_k
        Q   = R*S + (1-R)*X_k        -> Y[k] = Q
    returns (Y, Q_final[None], S_final[None])

Distribution: data-parallel over batch B=64 across 8 cores (8 per core).

Device layout ("transposed world"): every [B, HD] tensor lives as
[128 partitions, (h_tile, b) free] where h = 128*h_tile + p.  The per-step
GEMM uses stationary weight tiles (Wih^T [128,128] bf16 blocks) and streams
V^T [128, 8].  Gate biases are injected into PSUM by one extra matmul
(lhsT = bias reshaped [8,128] zero-padded, rhs = kron(I8, ones(1,8))), so
each gate bank needs a single sigmoid ACT op over [128, 64].
"""

import os
import sys

import numpy as np

for _p in ("/opt/trn_rl_repo", "/opt/pypackages"):
    if _p not in sys.path:
        sys.path.append(_p)

T, B, HD = 512, 64, 1024
ID = HD
NCORES = 8
BL = B // NCORES          # batch per core = 8
KT = ID // 128            # contraction tiles = 8
C = 16                    # recurrence chunk length (steps)
ACH = 64                  # A-phase chunk length (steps)

LAST_RESULTS = None       # BassKernelResults of the most recent run (for test.py)


def _build_program(t_steps=T):
    import concourse.bass as bass
    import concourse.mybir as mybir
    import concourse.tile as tile
    from concourse import bacc
    from contextlib import ExitStack

    f32 = mybir.dt.float32
    bf16 = mybir.dt.bfloat16
    SIG = mybir.ActivationFunctionType.Sigmoid
    COPY = mybir.ActivationFunctionType.Copy

    nc = bacc.Bacc("TRN2", target_bir_lowering=False, debug=False, num_devices=NCORES)

    x_p = nc.dram_tensor("x_p", [KT, 128, t_steps, BL], f32, kind="ExternalInput").ap()
    wih = nc.dram_tensor("wih", [128, KT, 2 * HD], bf16, kind="ExternalInput").ap()
    wa = nc.dram_tensor("wa", [128, KT, HD], f32, kind="ExternalInput").ap()
    gbv = nc.dram_tensor("gbv", [128, KT, BL], f32, kind="ExternalInput").ap()
    bf = nc.dram_tensor("bf", [128, 128], bf16, kind="ExternalInput").ap()
    br = nc.dram_tensor("br", [128, 128], bf16, kind="ExternalInput").ap()
    ind = nc.dram_tensor("ind", [128, KT * BL], bf16, kind="ExternalInput").ap()
    q0 = nc.dram_tensor("q0", [128, KT, BL], f32, kind="ExternalInput").ap()
    s0 = nc.dram_tensor("s0", [128, KT, BL], f32, kind="ExternalInput").ap()
    y_p = nc.dram_tensor("y_p", [KT, 128, t_steps, BL], f32, kind="ExternalOutput").ap()
    sf = nc.dram_tensor("sf", [128, KT, BL], f32, kind="ExternalOutput").ap()

    with tile.TileContext(nc) as tc, ExitStack() as ctx:
        constp = ctx.enter_context(tc.tile_pool(name="const", bufs=1))
        dramp = ctx.enter_context(tc.tile_pool(name="dram", bufs=1, space="DRAM"))
        xap = ctx.enter_context(tc.tile_pool(name="xa", bufs=2))
        apsum = ctx.enter_context(tc.tile_pool(name="apsum", bufs=2, space="PSUM"))
        acp = ctx.enter_context(tc.tile_pool(name="acopy", bufs=3))
        xrp = ctx.enter_context(tc.tile_pool(name="xr", bufs=2))
        arp = ctx.enter_context(tc.tile_pool(name="ar", bufs=2))
        yp = ctx.enter_context(tc.tile_pool(name="y", bufs=2))
        rpsum = ctx.enter_context(tc.tile_pool(name="rpsum", bufs=2, space="PSUM"))
        frp = ctx.enter_context(tc.tile_pool(name="fr", bufs=2))
        vp = ctx.enter_context(tc.tile_pool(name="v", bufs=2))
        tmpp = ctx.enter_context(tc.tile_pool(name="tmp", bufs=3))

        # ---- constants ----
        wih_sb = constp.tile([128, KT, 2 * HD], bf16)
        nc.sync.dma_start(wih_sb[:], wih)
        wa_sb = constp.tile([128, KT, HD], f32)
        nc.sync.dma_start(wa_sb[:], wa)
        gb_sb = constp.tile([128, KT, BL], f32)
        nc.sync.dma_start(gb_sb[:], gbv)
        bf_sb = constp.tile([128, 128], bf16)
        nc.sync.dma_start(bf_sb[:], bf)
        br_sb = constp.tile([128, 128], bf16)
        nc.sync.dma_start(br_sb[:], br)
        ind_sb = constp.tile([128, KT * BL], bf16)
        nc.sync.dma_start(ind_sb[:], ind)
        q0_sb = constp.tile([128, KT, BL], f32)
        nc.sync.dma_start(q0_sb[:], q0)
        S = constp.tile([128, KT, BL], f32)
        nc.sync.dma_start(S[:], s0)

        a_dram = dramp.tile([KT, 128, t_steps, BL], f32)

        # ---- Phase A: A^T = WA @ X^T  (big parallel GEMM) ----
        ach = min(ACH, t_steps)
        for ac in range(t_steps // ach):
            xc = xap.tile([128, KT, ach, BL], f32, tag="xa")
            nc.sync.dma_start(
                xc[:],
                x_p[:, :, ac * ach:(ac + 1) * ach, :].rearrange("i p k b -> p i k b"),
            )
            for m in range(KT):
                ps = apsum.tile([128, ach * BL], f32, tag="aps")
                for i in range(KT):
                    nc.tensor.matmul(
                        ps[:],
                        wa_sb[:, i, m * 128:(m + 1) * 128],
                        xc[:, i],
                        start=(i == 0),
                        stop=(i == KT - 1),
                    )
                asb = acp.tile([128, ach, BL], f32, tag="acp")
                nc.scalar.activation(asb.rearrange("p k b -> p (k b)"), ps[:], COPY)
                nc.sync.dma_start(a_dram[m, :, ac * ach:(ac + 1) * ach, :], asb[:])

        # ---- Phase R: the recurrence ----
        prev_q = q0_sb[:]
        for ch in range(t_steps // C):
            xc = xrp.tile([128, KT, C, BL], f32, tag="xr")
            nc.sync.dma_start(
                xc[:],
                x_p[:, :, ch * C:(ch + 1) * C, :].rearrange("i p k b -> p i k b"),
            )
            aq = arp.tile([128, KT, C, BL], f32, tag="ar")
            nc.sync.dma_start(
                aq[:],
                a_dram[:, :, ch * C:(ch + 1) * C, :].rearrange("i p k b -> p i k b"),
            )
            yc = yp.tile([128, KT, C, BL], f32, tag="y")
            for kk in range(C):
                Xk = xc[:, :, kk, :]          # [128, KT, BL] strided
                Ak = aq[:, :, kk, :]
                # V = Xk + gb * Q_prev
                V = vp.tile([128, KT, BL], bf16, tag="v")
                nc.vector.tensor_mul(V[:], gb_sb[:], prev_q)
                nc.vector.tensor_add(V[:], V[:], Xk)
                # D = S - A  (early; off the post-sigmoid critical path)
                D = tmpp.tile([128, KT, BL], f32, tag="d")
                nc.vector.tensor_sub(D[:], S[:], Ak)
                # gates GEMM: psF gets rows [0,1024), psR gets rows [1024,2048)
                psF = rpsum.tile([128, KT * BL], f32, tag="psF")
                psR = rpsum.tile([128, KT * BL], f32, tag="psR")
                nc.tensor.matmul(psF[:], bf_sb[:], ind_sb[:], start=True, stop=False,
                                 skip_group_check=True)
                for j in range(8):
                    for i in range(KT):
                        nc.tensor.matmul(
                            psF[:, j * BL:(j + 1) * BL],
                            wih_sb[:, i, j * 128:(j + 1) * 128],
                            V[:, i, :],
                            start=False,
                            stop=(j == 7 and i == KT - 1),
                            skip_group_check=True,
                        )
                F = frp.tile([128, KT * BL], f32, tag="F")
                nc.scalar.activation(F[:], psF[:], SIG)
                F3 = F.rearrange("p (i b) -> p i b", i=KT)
                # S = F*D + A
                nc.vector.tensor_mul(S[:], F3, D[:])
                nc.vector.tensor_add(S[:], S[:], Ak)
                # E = S - X (needs new S; still overlaps the R half of the GEMM)
                E = tmpp.tile([128, KT, BL], f32, tag="e")
                nc.vector.tensor_sub(E[:], S[:], Xk)
                nc.tensor.matmul(psR[:], br_sb[:], ind_sb[:], start=True, stop=False,
                                 skip_group_check=True)
                for j in range(8):
                    for i in range(KT):
                        nc.tensor.matmul(
                            psR[:, j * BL:(j + 1) * BL],
                            wih_sb[:, i, (8 + j) * 128:(9 + j) * 128],
                            V[:, i, :],
                            start=False,
                            stop=(j == 7 and i == KT - 1),
                            skip_group_check=True,
                        )
                R = frp.tile([128, KT * BL], f32, tag="R")
                nc.scalar.activation(R[:], psR[:], SIG)
                R3 = R.rearrange("p (i b) -> p i b", i=KT)
                # Q = R*E + X  -> written straight into the Y chunk
                Qs = yc[:, :, kk, :]
                nc.vector.tensor_mul(Qs, R3, E[:])
                nc.vector.tensor_add(Qs, Qs, Xk)
                prev_q = Qs
            nc.sync.dma_start(
                y_p[:, :, ch * C:(ch + 1) * C, :].rearrange("i p k b -> p i k b"),
                yc[:],
            )
        nc.sync.dma_start(sf, S[:])

    nc.compile()
    return nc


_PROGRAM_CACHE = {}


def _get_program(t_steps=T):
    if t_steps not in _PROGRAM_CACHE:
        _PROGRAM_CACHE[t_steps] = _build_program(t_steps)
    return _PROGRAM_CACHE[t_steps]


def kernel(x, Q0, S0, Wih, WA, uV_raw, uV2, PFB, HWb):
    global LAST_RESULTS
    from concourse.bass_utils import run_bass_kernel_spmd
    import ml_dtypes

    x = np.ascontiguousarray(np.asarray(x, dtype=np.float32))
    Q0 = np.asarray(Q0, dtype=np.float32)
    S0 = np.asarray(S0, dtype=np.float32)
    Wih = np.asarray(Wih, dtype=np.float32)
    WA = np.asarray(WA, dtype=np.float32)
    uV_raw = np.asarray(uV_raw, dtype=np.float32)
    uV2 = np.asarray(uV2, dtype=np.float32)
    PFB = np.asarray(PFB, dtype=np.float32)
    HWb = np.asarray(HWb, dtype=np.float32)

    t_steps = x.shape[0]

    # ---- host-side packing (pure layout, no FLOPs except the tiny [HD] gate) ----
    g = (1.0 / (1.0 + np.exp(-uV2, dtype=np.float64)) * np.tanh(uV_raw,
         dtype=np.float64)).astype(np.float32)                     # [HD]
    gb_packed = np.ascontiguousarray(
        np.broadcast_to(g.reshape(KT, 128).T[:, :, None], (128, KT, BL))
    ).astype(np.float32)

    wih_packed = np.ascontiguousarray(
        Wih.T.reshape(KT, 128, 2 * HD).transpose(1, 0, 2)
    ).astype(ml_dtypes.bfloat16)                                   # [128, KT, 2HD]
    wa_packed = np.ascontiguousarray(
        WA.T.reshape(KT, 128, HD).transpose(1, 0, 2)
    )                                                              # [128, KT, HD]

    bf_packed = np.zeros((128, 128), ml_dtypes.bfloat16)
    bf_packed[:KT] = PFB.reshape(KT, 128).astype(ml_dtypes.bfloat16)
    br_packed = np.zeros((128, 128), ml_dtypes.bfloat16)
    br_packed[:KT] = HWb.reshape(KT, 128).astype(ml_dtypes.bfloat16)
    ind_packed = np.zeros((128, KT * BL), ml_dtypes.bfloat16)
    ind_packed[:KT] = np.kron(np.eye(KT, dtype=np.float32), np.ones((1, BL), np.float32))

    shared = {
        "wih": wih_packed,
        "wa": wa_packed,
        "gbv": gb_packed,
        "bf": bf_packed,
        "br": br_packed,
        "ind": ind_packed,
    }

    in_maps = []
    for c in range(NCORES):
        bsl = slice(c * BL, (c + 1) * BL)
        x_core = x[:, bsl, :]                                      # [T, BL, HD]
        x_packed = np.ascontiguousarray(
            x_core.reshape(t_steps, BL, KT, 128).transpose(2, 3, 0, 1)
        )                                                          # [KT, 128, T, BL]
        q0_packed = np.ascontiguousarray(
            Q0[0, bsl, :].reshape(BL, KT, 128).transpose(2, 1, 0)
        )                                                          # [128, KT, BL]
        s0_packed = np.ascontiguousarray(
            S0[0, bsl, :].reshape(BL, KT, 128).transpose(2, 1, 0)
        )
        in_maps.append(dict(shared, x_p=x_packed, q0=q0_packed, s0=s0_packed))

    nc = _get_program(t_steps)
    trace = bool(int(os.environ.get("BK_TRACE", "0")))
    res = run_bass_kernel_spmd(nc, in_maps, core_ids=list(range(NCORES)), trace=trace)
    LAST_RESULTS = res

    # ---- unpack ----
    Y = np.empty((t_steps, B, HD), np.float32)
    Sf = np.empty((1, B, HD), np.float32)
    for c in range(NCORES):
        bsl = slice(c * BL, (c + 1) * BL)
        y_core = np.asarray(res.results[c]["y_p"])                 # [KT, 128, T, BL]
        Y[:, bsl, :] = y_core.transpose(2, 3, 0, 1).reshape(t_steps, BL, HD)
        sf_core = np.asarray(res.results[c]["sf"])                 # [128, KT, BL]
        Sf[0, bsl, :] = sf_core.transpose(2, 1, 0).reshape(BL, HD)
    Qf = Y[-1][None].copy()
    return Y, Qf, Sf
